# revision 5
# baseline (speedup 1.0000x reference)
"""HSIC loss kernel for 8 TRN2 NeuronCores.

Math: loss = -tr(CKW·CKG)/(n-1)^2 with CKX = KX·H, H = I - 1/n.
Expanded:  T  = S1 - (2/n)·Σ_i sW_i·sG_i + SW·SG/n²,  loss = -T/(n-1)²
where S1 = Σ_ij KW∘KG, sX = row sums of KX (KX symmetric).

The 2n×2n kernel matrix is only needed through its two diagonal blocks:
the cross blocks only enter via the bandwidth, and Σd2 has the closed
form 2N·Σsq - 2·||Σ_i x_i||², so bandwidth is computed on host.

Sharding: row-blocks of KW and KG. Core c computes rows [c·512, (c+1)·512)
of both 4096×4096 kernel blocks, reducing them on the fly to per-row
partial sums (Στ via ACT accum, Σ(τ²+τ⁴+τ⁸+τ¹⁶) via a custom DVE op,
Σ kW·kG via custom TENSOR_TENSOR_REDUCE). Host combines 8×[128,32]
partials in f64. No collectives needed.

Per out-tile [128,512]: PSUM = Σ_k WT[k,i]·WT[k,j] (4 bf16 matmuls)
+ (ã_i + ã_j) via one K=4 matmul with rows [1,1,ã_hi,ã_lo]/[ã_hi,ã_lo,1,1]
(ã = -sq/2 split hi/lo in bf16 so the add is f32-accurate), then
τ = Exp(P/(8bw)) on ACT = exp(-d2/(16bw)), and k = τ+τ²+τ⁴+τ⁸+τ¹⁶
= Σ_a exp(-d2/(bw·2^a)) via one custom DVE pass.
"""
import numpy as np
import ml_dtypes
from contextlib import ExitStack
from operator import add as _op_add

import concourse.bass as bass
import concourse.tile as tile
from concourse import bacc, mybir
import concourse.dve_ops as dve_ops
from concourse.dve_spec import Spec, Src0, Zero
from concourse.dve_ops import DveOp, _ref_body_sum

N_ROWS = 4096     # n
D = 512           # feature dim
NCORES = 8
ROWS_PER_CORE = N_ROWS // NCORES     # 512
P = 128
NM = ROWS_PER_CORE // P              # 4 row blocks per core
NJ = N_ROWS // 512                   # 8 column chunks of 512
NCOL = NJ * NM                       # 32 accum columns
KERNEL_NUM = 5
BF16 = ml_dtypes.bfloat16
LAST_RESULT = None
LAST_SCALE = None


def _ref_pows(in0, in1, c0, c1, c2):
    t = in0.astype(np.float32)
    t2 = t * t
    t4 = t2 * t2
    t8 = t4 * t4
    t16 = t8 * t8
    return (t2 + t4 + t8 + t16).astype(np.float32)


def _register_powsum():
    name = "POWSUM_HI_ANT"
    for op in dve_ops.OPS:
        if op.name == name:
            return op
    t = Src0
    t2 = t * t
    t4 = t2 * t2
    t8 = t4 * t4
    t16 = t8 * t8
    body = (t2 + t4) + (t8 + t16)
    spec = Spec(body=body, accum=_op_add, accum_init=Zero,
                reference=_ref_body_sum(_ref_pows))
    op = DveOp(name, spec, subdim=False,
               uops_sha={'v3': '250d8b54fc692992', 'v4': '05962d123e30a773'})
    dve_ops.OPS.append(op)
    dve_ops._SUB_OPCODE_FOR_NAME[name] = (
        dve_ops._CUSTOM_DVE_ROW_BASE + len(dve_ops.OPS) - 1)
    dve_ops.CUSTOM_DVE_SPECS[name] = op.spec
    return op


ADD_DVE_OF_8 = 2


def _build(scale: float):
    POWSUM = _register_powsum()
    f32 = mybir.dt.float32
    bf16 = mybir.dt.bfloat16
    nc = bacc.Bacc("TRN2", target_bir_lowering=False, debug=False)

    wt_d = nc.dram_tensor("wt", [D, N_ROWS], bf16, kind="ExternalInput")
    gt_d = nc.dram_tensor("gt", [D, N_ROWS], bf16, kind="ExternalInput")
    lw_d = nc.dram_tensor("lw", [D, ROWS_PER_CORE], bf16, kind="ExternalInput")
    lg_d = nc.dram_tensor("lg", [D, ROWS_PER_CORE], bf16, kind="ExternalInput")
    awr_d = nc.dram_tensor("awr", [4, N_ROWS], bf16, kind="ExternalInput")
    agr_d = nc.dram_tensor("agr", [4, N_ROWS], bf16, kind="ExternalInput")
    awl_d = nc.dram_tensor("awl", [4, ROWS_PER_CORE], bf16, kind="ExternalInput")
    agl_d = nc.dram_tensor("agl", [4, ROWS_PER_CORE], bf16, kind="ExternalInput")
    outs = {q: nc.dram_tensor(q, [P, NCOL], f32, kind="ExternalOutput")
            for q in ("acc_tw", "acc_sw", "acc_tg", "acc_sg", "acc_s1")}

    with tile.TileContext(nc) as tc, ExitStack() as ctx:
        const = ctx.enter_context(tc.tile_pool(name="const", bufs=1))
        rhsp = ctx.enter_context(tc.tile_pool(name="rhs", bufs=3))
        psum = ctx.enter_context(tc.tile_pool(name="psum", bufs=6, space="PSUM"))
        taup = ctx.enter_context(tc.tile_pool(name="tau", bufs=4))
        kp = ctx.enter_context(tc.tile_pool(name="kk", bufs=6))
        accp = ctx.enter_context(tc.tile_pool(name="acc", bufs=1))

        # persistent: lhsT slabs (4 partition blocks each), aug tiles, accum
        lw_t = [const.tile([P, ROWS_PER_CORE], bf16, tag=f"lw{kb}", name=f"lw{kb}") for kb in range(4)]
        lg_t = [const.tile([P, ROWS_PER_CORE], bf16, tag=f"lg{kb}", name=f"lg{kb}") for kb in range(4)]
        for kb in range(4):
            nc.sync.dma_start(lw_t[kb][:], lw_d.ap()[kb * P:(kb + 1) * P, :])
            nc.sync.dma_start(lg_t[kb][:], lg_d.ap()[kb * P:(kb + 1) * P, :])
        awr_t = const.tile([4, N_ROWS], bf16, tag="awr", name="awr_t")
        agr_t = const.tile([4, N_ROWS], bf16, tag="agr", name="agr_t")
        awl_t = const.tile([4, ROWS_PER_CORE], bf16, tag="awl", name="awl_t")
        agl_t = const.tile([4, ROWS_PER_CORE], bf16, tag="agl", name="agl_t")
        nc.sync.dma_start(awr_t[:], awr_d.ap()[:])
        nc.sync.dma_start(agr_t[:], agr_d.ap()[:])
        nc.sync.dma_start(awl_t[:], awl_d.ap()[:])
        nc.sync.dma_start(agl_t[:], agl_d.ap()[:])
        acc = {q: accp.tile([P, NCOL], f32, tag=q, name=q + "_t") for q in outs}
        for q in acc:
            nc.any.memset(acc[q][:], 0.0)

        for jc in range(NJ):
            rw = [rhsp.tile([P, 512], bf16, tag=f"rw{kb}", name=f"rw{kb}") for kb in range(4)]
            rg = [rhsp.tile([P, 512], bf16, tag=f"rg{kb}", name=f"rg{kb}") for kb in range(4)]
            for kb in range(4):
                nc.sync.dma_start(rw[kb][:], wt_d.ap()[kb * P:(kb + 1) * P,
                                                       jc * 512:(jc + 1) * 512])
                nc.sync.dma_start(rg[kb][:], gt_d.ap()[kb * P:(kb + 1) * P,
                                                       jc * 512:(jc + 1) * 512])
            for m in range(NM):
                col = jc * NM + m
                ktiles = {}
                for X, lhs, rhs, augl, augr in (("w", lw_t, rw, awl_t, awr_t),
                                                ("g", lg_t, rg, agl_t, agr_t)):
                    ps = psum.tile([P, 512], f32, tag="ps", name="ps")
                    for kb in range(4):
                        nc.tensor.matmul(ps[:], lhs[kb][:, m * P:(m + 1) * P],
                                         rhs[kb][:], start=(kb == 0), stop=False)
                    nc.tensor.matmul(ps[:], augl[:, m * P:(m + 1) * P],
                                     augr[:, jc * 512:(jc + 1) * 512],
                                     start=False, stop=True)
                    tau = taup.tile([P, 512], f32, tag="tau", name="tau")
                    nc.scalar.activation(tau[:], ps[:],
                                         mybir.ActivationFunctionType.Exp,
                                         bias=0.0, scale=scale,
                                         accum_out=acc["acc_t" + X][:, col:col + 1])
                    s = kp.tile([P, 512], f32, tag="s", name="s")
                    nc.vector._custom_dve(POWSUM, out=s[:], in0=tau[:],
                                          accum_out=acc["acc_s" + X][:, col:col + 1])
                    k = kp.tile([P, 512], f32, tag="k" + X, name="k" + X)
                    _ctr = jc * NM * 2 + m * 2 + (0 if X == "w" else 1)
                    if (_ctr % 8) < ADD_DVE_OF_8:
                        nc.vector.tensor_add(k[:], tau[:], s[:])
                    else:
                        nc.gpsimd.tensor_add(k[:], tau[:], s[:])
                    ktiles[X] = k
                dummy = kp.tile([P, 512], f32, tag="dummy", name="dummy")
                nc.vector._custom_dve(dve_ops.TENSOR_TENSOR_REDUCE, out=dummy[:],
                                      in0=ktiles["w"][:], in1=ktiles["g"][:],
                                      s0=0.0, s1=1.0,
                                      accum_out=acc["acc_s1"][:, col:col + 1])
        for q, d in outs.items():
            nc.sync.dma_start(d.ap()[:], acc[q][:])
    nc.compile()
    return nc


def _powsum5(t):
    t2 = t * t
    t4 = t2 * t2
    t8 = t4 * t4
    return t + t2 + t4 + t8 + t8 * t8


def kernel(W, G, **_):
    from concourse.bass_utils import run_bass_kernel_spmd
    W = np.asarray(W, dtype=np.float32)
    G = np.asarray(G, dtype=np.float32)
    n = W.shape[0]
    N = 2 * n

    # host prep (f64)
    W64, G64 = W.astype(np.float64), G.astype(np.float64)
    sqW = (W64 * W64).sum(1)
    sqG = (G64 * G64).sum(1)
    colsum = W64.sum(0) + G64.sum(0)
    sum_d2 = 2.0 * N * (sqW.sum() + sqG.sum()) - 2.0 * (colsum * colsum).sum()
    bw = sum_d2 / (N * N - N) / (2.0 ** (KERNEL_NUM // 2))
    scale = float(np.float32(1.0 / (8.0 * bw)))

    WTb = np.ascontiguousarray(W.T).astype(BF16)
    GTb = np.ascontiguousarray(G.T).astype(BF16)
    ones_row = np.ones(n, np.float64)

    def aug(sq):
        a = -0.5 * sq
        hi = a.astype(BF16)
        lo = (a - hi.astype(np.float64)).astype(BF16)
        return hi, lo
    awhi, awlo = aug(sqW)
    aghi, aglo = aug(sqG)
    awr = np.stack([awhi, awlo, ones_row.astype(BF16), ones_row.astype(BF16)])
    agr = np.stack([aghi, aglo, ones_row.astype(BF16), ones_row.astype(BF16)])

    global LAST_SCALE
    LAST_SCALE = scale
    nc = _build(scale)
    in_maps = []
    for c in range(NCORES):
        r0, r1 = c * ROWS_PER_CORE, (c + 1) * ROWS_PER_CORE
        o = np.ones(ROWS_PER_CORE, BF16)
        in_maps.append({
            "wt": WTb, "gt": GTb,
            "lw": np.ascontiguousarray(WTb[:, r0:r1]),
            "lg": np.ascontiguousarray(GTb[:, r0:r1]),
            "awr": awr, "agr": agr,
            "awl": np.stack([o, o, awhi[r0:r1], awlo[r0:r1]]),
            "agl": np.stack([o, o, aghi[r0:r1], aglo[r0:r1]]),
        })
    import os
    # NTFF profiling hook (antenv.axon_hooks) is absent in this container;
    # run_bass_kernel_spmd would crash resolving it if BASS_TRACE leaks in.
    os.environ["BASS_NEVER_TRACE"] = "1"
    res = run_bass_kernel_spmd(nc, in_maps, core_ids=list(range(NCORES)))
    global LAST_RESULT
    LAST_RESULT = res

    # host combine (f64)
    S1 = 0.0
    sW = np.zeros(n)
    sG = np.zeros(n)
    for c, out in enumerate(res.results):
        S1 += out["acc_s1"].astype(np.float64).sum()
        tw = out["acc_tw"].astype(np.float64) + out["acc_sw"].astype(np.float64)
        tg = out["acc_tg"].astype(np.float64) + out["acc_sg"].astype(np.float64)
        for m in range(NM):
            rows = slice(c * ROWS_PER_CORE + m * P, c * ROWS_PER_CORE + (m + 1) * P)
            sW[rows] = tw[:, m::NM].sum(1)
            sG[rows] = tg[:, m::NM].sum(1)

    # replace the (numerically noisy under bf16) diagonal with its exact value
    def diag_dev(Tb, ahi, alo):
        g_ii = (Tb.astype(np.float64) ** 2).sum(0)
        a2 = 2.0 * (ahi.astype(np.float64) + alo.astype(np.float64))
        return _powsum5(np.exp((g_ii + a2) * scale))
    kWd = diag_dev(WTb, awhi, awlo)
    kGd = diag_dev(GTb, aghi, aglo)
    S1 += (25.0 - kWd * kGd).sum()
    sW += 5.0 - kWd
    sG += 5.0 - kGd

    T = S1 - (2.0 / n) * (sW * sG).sum() + sW.sum() * sG.sum() / (n * n)
    loss = -T / ((n - 1) ** 2)
    return np.float32(loss)


# revision 6
# speedup vs baseline: 1.0182x; 1.0182x over previous
"""HSIC loss kernel for 8 TRN2 NeuronCores.

Math: loss = -tr(CKW·CKG)/(n-1)^2 with CKX = KX·H, H = I - 1/n.
Expanded:  T  = S1 - (2/n)·Σ_i sW_i·sG_i + SW·SG/n²,  loss = -T/(n-1)²
where S1 = Σ_ij KW∘KG, sX = row sums of KX (KX symmetric).

The 2n×2n kernel matrix is only needed through its two diagonal blocks:
the cross blocks only enter via the bandwidth, and Σd2 has the closed
form 2N·Σsq - 2·||Σ_i x_i||², so bandwidth is computed on host.

Sharding: row-blocks of KW and KG. Core c computes rows [c·512, (c+1)·512)
of both 4096×4096 kernel blocks, reducing them on the fly to per-row
partial sums (Στ via ACT accum, Σ(τ²+τ⁴+τ⁸+τ¹⁶) via a custom DVE op,
Σ kW·kG via custom TENSOR_TENSOR_REDUCE). Host combines 8×[128,32]
partials in f64. No collectives needed.

Per out-tile [128,512]: PSUM = Σ_k WT[k,i]·WT[k,j] (4 bf16 matmuls)
+ (ã_i + ã_j) via one K=4 matmul with rows [1,1,ã_hi,ã_lo]/[ã_hi,ã_lo,1,1]
(ã = -sq/2 split hi/lo in bf16 so the add is f32-accurate), then
τ = Exp(P/(8bw)) on ACT = exp(-d2/(16bw)), and k = τ+τ²+τ⁴+τ⁸+τ¹⁶
= Σ_a exp(-d2/(bw·2^a)) via one custom DVE pass.
"""
import numpy as np
import ml_dtypes
from contextlib import ExitStack
from operator import add as _op_add

import concourse.bass as bass
import concourse.tile as tile
from concourse import bacc, mybir
import concourse.dve_ops as dve_ops
from concourse.dve_spec import Spec, Src0, Zero
from concourse.dve_ops import DveOp, _ref_body_sum

N_ROWS = 4096     # n
D = 512           # feature dim
NCORES = 8
ROWS_PER_CORE = N_ROWS // NCORES     # 512
P = 128
NM = ROWS_PER_CORE // P              # 4 row blocks per core
NJ = N_ROWS // 512                   # 8 column chunks of 512
NCOL = NJ * NM                       # 32 accum columns
KERNEL_NUM = 5
BF16 = ml_dtypes.bfloat16
LAST_RESULT = None
LAST_SCALE = None


def _ref_pows(in0, in1, c0, c1, c2):
    t = in0.astype(np.float32)
    t2 = t * t
    t4 = t2 * t2
    t8 = t4 * t4
    t16 = t8 * t8
    return (t2 + t4 + t8 + t16).astype(np.float32)


def _register_powsum():
    name = "POWSUM_HI_ANT"
    for op in dve_ops.OPS:
        if op.name == name:
            return op
    t = Src0
    t2 = t * t
    t4 = t2 * t2
    t8 = t4 * t4
    t16 = t8 * t8
    body = (t2 + t4) + (t8 + t16)
    spec = Spec(body=body, accum=_op_add, accum_init=Zero,
                reference=_ref_body_sum(_ref_pows))
    op = DveOp(name, spec, subdim=False,
               uops_sha={'v3': '250d8b54fc692992', 'v4': '05962d123e30a773'})
    dve_ops.OPS.append(op)
    dve_ops._SUB_OPCODE_FOR_NAME[name] = (
        dve_ops._CUSTOM_DVE_ROW_BASE + len(dve_ops.OPS) - 1)
    dve_ops.CUSTOM_DVE_SPECS[name] = op.spec
    return op


ADD_DVE_OF_8 = 2


def _build(scale: float):
    POWSUM = _register_powsum()
    f32 = mybir.dt.float32
    bf16 = mybir.dt.bfloat16
    nc = bacc.Bacc("TRN2", target_bir_lowering=False, debug=False)

    wt_d = nc.dram_tensor("wt", [D, N_ROWS], bf16, kind="ExternalInput")
    gt_d = nc.dram_tensor("gt", [D, N_ROWS], bf16, kind="ExternalInput")
    lw_d = nc.dram_tensor("lw", [D, ROWS_PER_CORE], bf16, kind="ExternalInput")
    lg_d = nc.dram_tensor("lg", [D, ROWS_PER_CORE], bf16, kind="ExternalInput")
    awr_d = nc.dram_tensor("awr", [4, N_ROWS], bf16, kind="ExternalInput")
    agr_d = nc.dram_tensor("agr", [4, N_ROWS], bf16, kind="ExternalInput")
    awl_d = nc.dram_tensor("awl", [4, ROWS_PER_CORE], bf16, kind="ExternalInput")
    agl_d = nc.dram_tensor("agl", [4, ROWS_PER_CORE], bf16, kind="ExternalInput")
    outs = {q: nc.dram_tensor(q, [P, NCOL], f32, kind="ExternalOutput")
            for q in ("acc_tw", "acc_sw", "acc_tg", "acc_sg", "acc_s1")}

    with tile.TileContext(nc) as tc, ExitStack() as ctx:
        const = ctx.enter_context(tc.tile_pool(name="const", bufs=1))
        rhsp = ctx.enter_context(tc.tile_pool(name="rhs", bufs=3))
        psum = ctx.enter_context(tc.tile_pool(name="psum", bufs=6, space="PSUM"))
        taup = ctx.enter_context(tc.tile_pool(name="tau", bufs=4))
        kp = ctx.enter_context(tc.tile_pool(name="kk", bufs=6))
        accp = ctx.enter_context(tc.tile_pool(name="acc", bufs=1))

        # persistent: lhsT slabs (4 partition blocks each), aug tiles, accum
        lw_t = [const.tile([P, ROWS_PER_CORE], bf16, tag=f"lw{kb}", name=f"lw{kb}") for kb in range(4)]
        lg_t = [const.tile([P, ROWS_PER_CORE], bf16, tag=f"lg{kb}", name=f"lg{kb}") for kb in range(4)]
        for kb in range(4):
            nc.sync.dma_start(lw_t[kb][:], lw_d.ap()[kb * P:(kb + 1) * P, :])
            nc.sync.dma_start(lg_t[kb][:], lg_d.ap()[kb * P:(kb + 1) * P, :])
        awr_t = const.tile([4, N_ROWS], bf16, tag="awr", name="awr_t")
        agr_t = const.tile([4, N_ROWS], bf16, tag="agr", name="agr_t")
        awl_t = const.tile([4, ROWS_PER_CORE], bf16, tag="awl", name="awl_t")
        agl_t = const.tile([4, ROWS_PER_CORE], bf16, tag="agl", name="agl_t")
        nc.sync.dma_start(awr_t[:], awr_d.ap()[:])
        nc.sync.dma_start(agr_t[:], agr_d.ap()[:])
        nc.sync.dma_start(awl_t[:], awl_d.ap()[:])
        nc.sync.dma_start(agl_t[:], agl_d.ap()[:])
        acc = {q: accp.tile([P, NCOL], f32, tag=q, name=q + "_t") for q in outs}

        for jc in range(NJ):
            rw = [rhsp.tile([P, 512], bf16, tag=f"rw{kb}", name=f"rw{kb}") for kb in range(4)]
            rg = [rhsp.tile([P, 512], bf16, tag=f"rg{kb}", name=f"rg{kb}") for kb in range(4)]
            for kb in range(4):
                nc.sync.dma_start(rw[kb][:], wt_d.ap()[kb * P:(kb + 1) * P,
                                                       jc * 512:(jc + 1) * 512])
                nc.sync.dma_start(rg[kb][:], gt_d.ap()[kb * P:(kb + 1) * P,
                                                       jc * 512:(jc + 1) * 512])
            for m in range(NM):
                col = jc * NM + m
                ktiles = {}
                for X, lhs, rhs, augl, augr in (("w", lw_t, rw, awl_t, awr_t),
                                                ("g", lg_t, rg, agl_t, agr_t)):
                    ps = psum.tile([P, 512], f32, tag="ps", name="ps")
                    for kb in range(4):
                        nc.tensor.matmul(ps[:], lhs[kb][:, m * P:(m + 1) * P],
                                         rhs[kb][:], start=(kb == 0), stop=False)
                    nc.tensor.matmul(ps[:], augl[:, m * P:(m + 1) * P],
                                     augr[:, jc * 512:(jc + 1) * 512],
                                     start=False, stop=True)
                    tau = taup.tile([P, 512], f32, tag="tau", name="tau")
                    nc.scalar.activation(tau[:], ps[:],
                                         mybir.ActivationFunctionType.Exp,
                                         bias=0.0, scale=scale,
                                         accum_out=acc["acc_t" + X][:, col:col + 1])
                    s = kp.tile([P, 512], f32, tag="s", name="s")
                    nc.vector._custom_dve(POWSUM, out=s[:], in0=tau[:],
                                          accum_out=acc["acc_s" + X][:, col:col + 1])
                    k = kp.tile([P, 512], f32, tag="k" + X, name="k" + X)
                    _ctr = jc * NM * 2 + m * 2 + (0 if X == "w" else 1)
                    if (_ctr % 8) < ADD_DVE_OF_8:
                        nc.vector.tensor_add(k[:], tau[:], s[:])
                    else:
                        nc.gpsimd.tensor_add(k[:], tau[:], s[:])
                    ktiles[X] = k
                dummy = kp.tile([P, 512], f32, tag="dummy", name="dummy")
                nc.vector._custom_dve(dve_ops.TENSOR_TENSOR_REDUCE, out=dummy[:],
                                      in0=ktiles["w"][:], in1=ktiles["g"][:],
                                      s0=0.0, s1=1.0,
                                      accum_out=acc["acc_s1"][:, col:col + 1])
        for q, d in outs.items():
            nc.sync.dma_start(d.ap()[:], acc[q][:])
    nc.compile()
    return nc


def _powsum5(t):
    t2 = t * t
    t4 = t2 * t2
    t8 = t4 * t4
    return t + t2 + t4 + t8 + t8 * t8


def kernel(W, G, **_):
    from concourse.bass_utils import run_bass_kernel_spmd
    W = np.asarray(W, dtype=np.float32)
    G = np.asarray(G, dtype=np.float32)
    n = W.shape[0]
    N = 2 * n

    # host prep (f64)
    W64, G64 = W.astype(np.float64), G.astype(np.float64)
    sqW = (W64 * W64).sum(1)
    sqG = (G64 * G64).sum(1)
    colsum = W64.sum(0) + G64.sum(0)
    sum_d2 = 2.0 * N * (sqW.sum() + sqG.sum()) - 2.0 * (colsum * colsum).sum()
    bw = sum_d2 / (N * N - N) / (2.0 ** (KERNEL_NUM // 2))
    scale = float(np.float32(1.0 / (8.0 * bw)))

    WTb = np.ascontiguousarray(W.T).astype(BF16)
    GTb = np.ascontiguousarray(G.T).astype(BF16)
    ones_row = np.ones(n, np.float64)

    def aug(sq):
        a = -0.5 * sq
        hi = a.astype(BF16)
        lo = (a - hi.astype(np.float64)).astype(BF16)
        return hi, lo
    awhi, awlo = aug(sqW)
    aghi, aglo = aug(sqG)
    awr = np.stack([awhi, awlo, ones_row.astype(BF16), ones_row.astype(BF16)])
    agr = np.stack([aghi, aglo, ones_row.astype(BF16), ones_row.astype(BF16)])

    global LAST_SCALE
    LAST_SCALE = scale
    nc = _build(scale)
    in_maps = []
    for c in range(NCORES):
        r0, r1 = c * ROWS_PER_CORE, (c + 1) * ROWS_PER_CORE
        o = np.ones(ROWS_PER_CORE, BF16)
        in_maps.append({
            "wt": WTb, "gt": GTb,
            "lw": np.ascontiguousarray(WTb[:, r0:r1]),
            "lg": np.ascontiguousarray(GTb[:, r0:r1]),
            "awr": awr, "agr": agr,
            "awl": np.stack([o, o, awhi[r0:r1], awlo[r0:r1]]),
            "agl": np.stack([o, o, aghi[r0:r1], aglo[r0:r1]]),
        })
    import os
    # NTFF profiling hook (antenv.axon_hooks) is absent in this container;
    # run_bass_kernel_spmd would crash resolving it if BASS_TRACE leaks in.
    os.environ["BASS_NEVER_TRACE"] = "1"
    res = run_bass_kernel_spmd(nc, in_maps, core_ids=list(range(NCORES)))
    global LAST_RESULT
    LAST_RESULT = res

    # host combine (f64)
    S1 = 0.0
    sW = np.zeros(n)
    sG = np.zeros(n)
    for c, out in enumerate(res.results):
        S1 += out["acc_s1"].astype(np.float64).sum()
        tw = out["acc_tw"].astype(np.float64) + out["acc_sw"].astype(np.float64)
        tg = out["acc_tg"].astype(np.float64) + out["acc_sg"].astype(np.float64)
        for m in range(NM):
            rows = slice(c * ROWS_PER_CORE + m * P, c * ROWS_PER_CORE + (m + 1) * P)
            sW[rows] = tw[:, m::NM].sum(1)
            sG[rows] = tg[:, m::NM].sum(1)

    # replace the (numerically noisy under bf16) diagonal with its exact value
    def diag_dev(Tb, ahi, alo):
        g_ii = (Tb.astype(np.float64) ** 2).sum(0)
        a2 = 2.0 * (ahi.astype(np.float64) + alo.astype(np.float64))
        return _powsum5(np.exp((g_ii + a2) * scale))
    kWd = diag_dev(WTb, awhi, awlo)
    kGd = diag_dev(GTb, aghi, aglo)
    S1 += (25.0 - kWd * kGd).sum()
    sW += 5.0 - kWd
    sG += 5.0 - kGd

    T = S1 - (2.0 / n) * (sW * sG).sum() + sW.sum() * sG.sum() / (n * n)
    loss = -T / ((n - 1) ** 2)
    return np.float32(loss)
